# revision 1
# baseline (speedup 1.0000x reference)
"""Trainium2 Bass kernel for nn_Conv2DLayer_16011638080159.

Math: out = C * (x @ weight.sum(0))   with x [524288, 512], weight [9, 512].
Equivalent to a row-wise dot product of x with w_eff = C * weight.sum(0).

Strategy (pure data parallel, per sharding hint):
  - Shard x along the batch axis across 8 NeuronCores (65536 rows each).
  - Host-side prep: fold the tiny K=9 weight sum and the C scale into a
    single [C] vector, replicated to a [128, 8*C] SBUF-ready constant.
  - Per core: stream x in [128 partitions, 8 rows x 512] tiles from HBM
    with 6-deep buffering, alternating the two HWDGE rings. The kernel is
    HBM bound (~415 us/core pure-DMA floor measured at 8 cores), so the
    row-dot-products are split so each compute engine stays below that:
      * Vector engine: fp32 tensor_tensor multiply of the whole tile by
        the replicated weight (1x mode), plus a segmented tensor_reduce
        for 1 of the 8 rows  (~320 us/core busy).
      * Scalar engine: the other 7 rows via ACTIVATE(Copy, accum_out),
        which sums 512 elems/row at 1 elem/cycle (~355 us/core busy).
  - Row mapping: shard row (p*512 + t*R + r) sits at partition p, tile t,
    slot r, so the per-core result tile [128, 512] is exactly the row-major
    view of the per-core output [65536]; one contiguous DMA writes it out.
"""

import numpy as np

import concourse.bacc as bacc
import concourse.bass as bass
import concourse.tile as tile
from concourse import mybir
from concourse.bass_utils import run_bass_kernel_spmd

B = 524288        # total rows
C = 512           # row length
N_CORES = 8
BS = B // N_CORES  # 65536 rows per core
P = 128            # SBUF partitions
RPP = BS // P      # 512 rows per partition
R = 8              # rows per partition per tile
F = R * C          # 4096 free elems per tile
NT = RPP // R      # 64 tiles per core
K_DVE = 1          # rows per tile reduced on DVE via segmented tensor_reduce

_NC_CACHE = None
LAST_RESULT = None  # BassKernelResults of the most recent run (for profiling)


def _build() -> bass.Bass:
    # Bacc (not raw Bass): its compile() pass splits multi-sem waits into
    # EventSemaphore instructions -- the TRN2 ISA allows only 1 wait/inst.
    nc = bacc.Bacc(None, target_bir_lowering=False, debug=False)
    x = nc.dram_tensor("x", [BS, C], mybir.dt.float32, kind="ExternalInput")
    w = nc.dram_tensor("w", [P, F], mybir.dt.float32, kind="ExternalInput")
    out = nc.dram_tensor("out", [BS], mybir.dt.float32, kind="ExternalOutput")

    # shard row (p*RPP + t*R + r) -> partition p, tile t, free slot (r, c)
    xv = x.rearrange("(p t r) c -> t p (r c)", p=P, t=NT, r=R)
    ov = out.rearrange("(p f) -> p f", p=P)

    n_act = R - K_DVE  # rows per tile reduced on the Scalar engine

    with tile.TileContext(nc) as tc:
        with (
            tc.tile_pool(name="const", bufs=1) as cpool,
            tc.tile_pool(name="xs", bufs=6) as xs,
            tc.tile_pool(name="ys", bufs=4) as ys,
            tc.tile_pool(name="scr", bufs=2) as scr,
            tc.tile_pool(name="res", bufs=1) as res,
        ):
            w_t = cpool.tile([P, F], mybir.dt.float32)
            nc.sync.dma_start(out=w_t[:], in_=w[:, :])
            o_t = res.tile([P, RPP], mybir.dt.float32)
            for t in range(NT):
                # All x DMAs go on the SP HWDGE ring: SP has no compute, so
                # DMA issue is never queued behind engine work (issuing from
                # nc.scalar stalls the DMA behind pending ACTIVATEs).
                x_t = xs.tile([P, F], mybir.dt.float32)
                nc.sync.dma_start(out=x_t[:], in_=xv[t])

                # one fp32 TT multiply for the whole tile
                y_t = ys.tile([P, F], mybir.dt.float32)
                nc.vector.tensor_mul(y_t[:], x_t[:], w_t[:])

                # ACT accumulates rows K_DVE..R-1 (one 512-sum per row)
                for r in range(n_act):
                    s_t = scr.tile([P, C], mybir.dt.float32, tag="act_s")
                    col = t * R + K_DVE + r
                    nc.scalar.activation(
                        out=s_t[:],
                        in_=y_t[:, (K_DVE + r) * C:(K_DVE + r + 1) * C],
                        func=mybir.ActivationFunctionType.Copy,
                        accum_out=o_t[:, col: col + 1],
                    )

                # DVE reduces rows 0..K_DVE-1 in one segmented reduce
                nc.vector.tensor_reduce(
                    out=o_t[:, t * R: t * R + K_DVE],
                    in_=y_t[:, 0:K_DVE * C].rearrange("p (r c) -> p r c", c=C),
                    axis=mybir.AxisListType.X,
                    op=mybir.AluOpType.add,
                )
            nc.sync.dma_start(out=ov, in_=o_t[:])
    nc.finalize()
    return nc


def kernel(x: np.ndarray, weight: np.ndarray) -> np.ndarray:
    global _NC_CACHE, LAST_RESULT
    x = np.ascontiguousarray(np.asarray(x), dtype=np.float32)
    weight = np.asarray(weight, dtype=np.float32)

    w_eff = (C * weight.sum(axis=0)).astype(np.float32)   # [C]
    w_rep = np.ascontiguousarray(np.tile(w_eff, (P, R)))  # [P, F]

    if _NC_CACHE is None:
        _NC_CACHE = _build()

    in_maps = [
        {"x": x[i * BS:(i + 1) * BS], "w": w_rep} for i in range(N_CORES)
    ]
    LAST_RESULT = run_bass_kernel_spmd(
        _NC_CACHE, in_maps, core_ids=list(range(N_CORES))
    )
    return np.concatenate([r["out"] for r in LAST_RESULT.results])

